# revision 52
# baseline (speedup 1.0000x reference)
"""Trainium2 Bass kernel for causal multi-head attention.

Problem: B=4, S=2048, D=512, H=8 heads (head_dim 64), causal mask.
  q = x @ Wq.T + bq ; k = x @ Wk.T + bk ; v = x @ Wv.T + bv
  att = softmax(mask(q k^T / sqrt(64))) @ v ; out = att @ Wo.T + bo
(bk drops out of softmax -- q.bk is constant across keys.)

Sharding: 8 cores = (batch b in 0..3) x (head-group hg in 0..1, 4 heads each).
Each core computes its 4 heads' Q/K/V projections, attention, and a partial
out-projection; host sums the two partials per batch and adds bo.

Device-side structure (causal fast path):
 - scores are computed TRANSPOSED (ST[k, q]) so exp(ST) feeds the attn*V
   matmul directly; softmax denominator comes from a ones-column interleaved
   in V (no reductions).
 - attention runs in 1-k-tile groups: the two heads of a pair are issued
   back-to-back at tile_position (0,0)/(64,0) so the 64-contract score
   matmuls co-run on the PE array, land in one double-buffered 2-bank PSUM
   tile, and ONE merged exp (strided [P,2,w] AP) covers both heads.  The
   double buffering lets the Act engine (exp, the pacing engine) pipeline
   with the PE instead of serializing group-by-group.
 - projections for s-block sb+1 and the out-projection of q-block qb-1 are
   woven as fillers between attention groups of q-block qb, so the PE's
   idle slots (while exp catches up) do the projection work and the Act
   engine never starves.
 - softmax normalization: both heads' sumexp rows sit in one PSUM row
   ([1, 1024]); reciprocal_approx_fast on DVE reads it straight from PSUM,
   gpsimd broadcasts the two halves, two DVE muls write the normalized
   attention (bf16) reading attention PSUM directly.
 - causal structure exact: k-tiles above the diagonal skipped, band tiles
   produce only their valid q columns, one [128,128] 0/1 triangle masks the
   leading block of each band tile.
 - matmul operands bf16; accumulation fp32 in PSUM.

The mask input is verified on the host: if it is exactly the causal mask the
fast path runs; otherwise a generic variant runs that reads a host-prepared
transposed multiplicative mask from DRAM.
"""

import sys

import numpy as np

for _p in ("/opt/trn_rl_repo",):
    if _p not in sys.path:
        sys.path.insert(0, _p)

import ml_dtypes  # noqa: E402

import concourse.bass as bass  # noqa: E402
import concourse.tile as tile  # noqa: E402
from concourse import bacc, mybir  # noqa: E402

B, S, D, H = 4, 2048, 512, 8
HD = D // H  # 64
P = 128
HG = 4  # heads per core
DG = HG * HD  # 256 per-core head dims
QB = 512  # q-block
NQB = S // QB  # 4
NKT = S // P  # 16 k-tiles
KTQ = QB // P  # 4 k-tiles per q-block (diagonal band width)
NET = D // P  # 4 e-tiles (contraction tiles for projections)
VW = HG * (HD + 1)  # 260: V with an interleaved ones-column per head
DELAY = 8  # attV drain lag in groups

F32 = mybir.dt.float32
BF16 = mybir.dt.bfloat16
NPBF16 = ml_dtypes.bfloat16

_BUILT = {}


def _build_nc_causal():
    nc = bacc.Bacc("TRN2", target_bir_lowering=False, debug=False, num_devices=8)

    xT_d = nc.dram_tensor("xT", [D + 1, S], BF16, kind="ExternalInput").ap()
    wq_d = nc.dram_tensor("wq", [D, DG], BF16, kind="ExternalInput").ap()
    bq_d = nc.dram_tensor("bqv", [DG, 1], F32, kind="ExternalInput").ap()
    wk_d = nc.dram_tensor("wk", [D, DG], BF16, kind="ExternalInput").ap()
    wv_d = nc.dram_tensor("wv", [D + 1, VW], BF16, kind="ExternalInput").ap()
    wo_d = nc.dram_tensor("wo", [DG, D], BF16, kind="ExternalInput").ap()
    bm_d = nc.dram_tensor("bm", [P, 2 * P], BF16, kind="ExternalInput").ap()
    out_d = nc.dram_tensor("out", [D, S], F32, kind="ExternalOutput").ap()

    EXP = mybir.ActivationFunctionType.Exp

    with tile.TileContext(nc) as tc:
        with (
            tc.tile_pool(name="consts", bufs=1) as consts,
            tc.tile_pool(name="work", bufs=11) as work,
            tc.tile_pool(name="attn", bufs=2) as attnp,
            tc.tile_pool(name="small", bufs=4) as small,
            tc.tile_pool(name="pmm", bufs=2, space="PSUM") as pmm,
            tc.tile_pool(name="pst", bufs=2, space="PSUM") as pst,
            tc.tile_pool(name="patt", bufs=1, space="PSUM") as patt,
        ):
            # ---- persistent SBUF tensors ----
            xts = [
                consts.tile([P, S], BF16, tag=f"xt{et}", name=f"xts{et}")
                for et in range(NET)
            ]
            wk_t = [
                consts.tile([P, DG], BF16, tag=f"wk{et}", name=f"wk{et}")
                for et in range(NET)
            ]
            wq_t = [
                consts.tile([P, DG], BF16, tag=f"wq{et}", name=f"wq{et}")
                for et in range(NET)
            ]
            bq_sb = [
                consts.tile([P, 1], F32, tag=f"bq{j}", name=f"bq{j}") for j in range(2)
            ]
            wv_t = [
                consts.tile([P, VW], BF16, tag=f"wv{et}", name=f"wv{et}")
                for et in range(NET)
            ]
            wvb = consts.tile([1, VW], BF16, tag="wvb", name="wvb")
            bvb = consts.tile([P, VW], BF16, tag="bvb", name="bvb")
            wo_t = [
                consts.tile([P, D], BF16, tag=f"wo{j}", name=f"wo{j}") for j in range(2)
            ]
            bm2 = consts.tile([P, 2 * P], BF16, tag="bm2", name="bm2")
            bm3 = bm2.rearrange("p (b c) -> p b c", c=P)

            QT = [consts.tile([P, S], BF16, tag=f"qt{i}", name=f"QT{i}") for i in range(2)]
            KT = [consts.tile([P, S], BF16, tag=f"kt{i}", name=f"KT{i}") for i in range(2)]
            V = [
                consts.tile([P, VW], BF16, tag=f"v{st}", name=f"Vt{st}")
                for st in range(NKT)
            ]

            # ---- PE warm-up: one accumulating dummy matmul chain (no
            # inter-matmul semaphores) keeps the PE busy through the DMA
            # head so the HAM clock-gate reaches K=8/8 before the
            # (PE-bound) projection phase ----
            dmy = consts.tile([P, QB], BF16, tag="dmy", name="dmy")
            nc.gpsimd.memset(dmy, 0)
            dsink = work.tile([P, QB], F32, tag="dsink", name="dsink")
            pd = pmm.tile([P, QB], F32, tag="mm", name="warm")
            for i in range(40):
                nc.tensor.matmul(pd, dmy[:, 0:P], dmy, start=(i == 0), stop=(i == 39))
            nc.vector.tensor_copy(dsink, pd)

            # ---- input DMAs, critical-first, spread over three queues ----
            # sync: wk (first K proj), wq, triangle mask, wo, x s-block 1
            for et in range(NET):
                nc.sync.dma_start(out=wk_t[et], in_=wk_d[et * P : (et + 1) * P, :])
            for et in range(NET):
                nc.sync.dma_start(out=wq_t[et], in_=wq_d[et * P : (et + 1) * P, :])
            nc.sync.dma_start(out=bm2, in_=bm_d)
            for j in range(2):
                nc.sync.dma_start(out=wo_t[j], in_=wo_d[j * P : (j + 1) * P, :])
            # gpsimd: x s-block 0 (first, one tile via scalar), wv, x 2-3
            nc.scalar.dma_start(out=xts[2][:, 0:QB], in_=xT_d[2 * P : 3 * P, 0:QB])
            for et in (0, 1, 3):
                nc.gpsimd.dma_start(
                    out=xts[et][:, 0:QB], in_=xT_d[et * P : (et + 1) * P, 0:QB]
                )
            for et in range(NET):
                nc.gpsimd.dma_start(out=wv_t[et], in_=wv_d[et * P : (et + 1) * P, :])
            nc.gpsimd.dma_start(out=wvb, in_=wv_d[D : D + 1, :])
            nc.gpsimd.partition_broadcast(bvb, wvb)
            # scalar: bq only (keep the exp queue clear)
            for j in range(2):
                nc.scalar.dma_start(out=bq_sb[j], in_=bq_d[j * P : (j + 1) * P, :])
            for sb, eng in ((1, nc.sync), (2, nc.gpsimd), (3, nc.gpsimd)):
                ssl = slice(sb * QB, (sb + 1) * QB)
                for et in range(NET):
                    eng.dma_start(
                        out=xts[et][:, ssl], in_=xT_d[et * P : (et + 1) * P, ssl]
                    )

            # ---- unit emitters ----
            def proj_units(sb):
                """Projection units for s-block sb: K/Q per head-half + 4 V tiles."""
                ssl = slice(sb * QB, (sb + 1) * QB)
                units = []
                for dgt in range(2):
                    dsl = slice(dgt * P, (dgt + 1) * P)

                    def ku(dgt=dgt, dsl=dsl):
                        ps = pmm.tile([P, QB], F32, tag="mm", name=f"psk{sb}{dgt}")
                        for et in range(NET):
                            nc.tensor.matmul(
                                ps,
                                wk_t[et][:, dsl],
                                xts[et][:, ssl],
                                start=(et == 0),
                                stop=(et == NET - 1),
                            )
                        nc.vector.tensor_copy(KT[dgt][:, ssl], ps)

                    def qu(dgt=dgt, dsl=dsl):
                        ps = pmm.tile([P, QB], F32, tag="mm", name=f"psq{sb}{dgt}")
                        for et in range(NET):
                            nc.tensor.matmul(
                                ps,
                                wq_t[et][:, dsl],
                                xts[et][:, ssl],
                                start=(et == 0),
                                stop=(et == NET - 1),
                            )
                        nc.vector.tensor_scalar_add(QT[dgt][:, ssl], ps, bq_sb[dgt])

                    units.append(ku)
                    units.append(qu)
                for st in range(4 * sb, 4 * sb + 4):
                    def vu(st=st):
                        ksl = slice(st * P, (st + 1) * P)
                        ps = pmm.tile([P, VW], F32, tag="mm", name=f"psv{st}")
                        for et in range(NET):
                            nc.tensor.matmul(
                                ps,
                                xts[et][:, ksl],
                                wv_t[et],
                                start=(et == 0),
                                stop=(et == NET - 1),
                            )
                        nc.vector.tensor_add(V[st], ps, bvb)

                    units.append(vu)
                return units

            attn_tiles = {}

            def outproj_units(qb):
                """Out-projection units for q-block qb (both head-halves)."""
                qsl = slice(qb * QB, (qb + 1) * QB)
                t0, t1 = attn_tiles[qb]
                units = []
                for et in range(NET):
                    def ou(et=et):
                        esl = slice(et * P, (et + 1) * P)
                        ops = pmm.tile([P, QB], F32, tag="mm", name=f"pso{qb}{et}")
                        nc.tensor.matmul(ops, wo_t[0][:, esl], t0, start=True, stop=False)
                        nc.tensor.matmul(ops, wo_t[1][:, esl], t1, start=False, stop=True)
                        ost = small.tile([P, QB], F32, tag="ost", name=f"ost{qb}{et}")
                        nc.vector.tensor_copy(ost, ops)
                        nc.sync.dma_start(out=out_d[esl, qsl], in_=ost)

                    units.append(ou)
                return units

            # ---- main interleaved loop ----
            carry = []  # [(attv_fn, g)...,(None, finish_fn)] from prev pass
            finished = set()
            units = {sb: proj_units(sb) for sb in range(NQB)}

            for qb in range(NQB):
                lastq = qb == NQB - 1
                # fillers by need-time: this block's own late units (dgt1 K/Q
                # needed at its second head-pair, V tiles needed by its attV
                # drains) first, then only the NEXT block's dgt0 K/Q (the
                # sole prerequisite of its first scores), then out-proj of
                # the previous block.  This moves most projection work out of
                # the PE-bound early phase into the Act-bound late phase.
                fillers = []
                if qb == 0:
                    units[0][0]()
                    units[0][1]()
                fillers += [("p", u) for u in units[qb][2:]]
                if qb + 1 < NQB:
                    fillers += [("p", u) for u in units[qb + 1][0:2]]
                if qb > 0:
                    fillers += [("o", u) for u in outproj_units(qb - 1)]
                nkt = KTQ * qb + KTQ
                ngroups = 2 * nkt
                acc = 0.0
                step = len(fillers) / ngroups if ngroups else 0.0

                def maybe_filler():
                    nonlocal acc
                    acc += step
                    while acc >= 1.0 and fillers:
                        kind, u = fillers[0]
                        if kind == "o" and (qb - 1, 1) not in finished:
                            break  # out-proj needs the previous attn_t written
                        fillers.pop(0)
                        u()
                        acc -= 1.0

                for hp in range(2):
                    dgt = hp
                    hA, hB = 2 * hp, 2 * hp + 1
                    rA, rB = slice(0, HD), slice(HD, 2 * HD)
                    attn_t = attnp.tile(
                        [P, QB], BF16, tag=f"attn{dgt}", name=f"attn{dgt}_{qb}"
                    )
                    if hp == 0:
                        attn_tiles[qb] = [attn_t, None]
                    else:
                        attn_tiles[qb][1] = attn_t
                    state = {}

                    def attv(g, state=state, hA=hA, hB=hB, nkt=nkt, qb=qb, hp=hp):
                        kt, ex, qo, w = g
                        # lazy allocation: by the first attV of this pair, the
                        # previous pair's normalize has been emitted, so the
                        # pool sees every access it must order against
                        if "attps" not in state:
                            state["attps"] = patt.tile(
                                [P, 2 * QB], F32, tag="att", name=f"att{qb}{hp}"
                            )
                        attps = state["attps"]
                        for h, hoff in ((hA, 0), (hB, QB)):
                            nc.tensor.matmul(
                                attps[0 : HD + 1, hoff + qo : hoff + qo + w],
                                V[kt][:, h * (HD + 1) : (h + 1) * (HD + 1)],
                                ex[:, hoff : hoff + w],
                                start=(kt == 0),
                                stop=(kt == nkt - 1),
                            )

                    def finish(state=state, qb=qb, hp=hp, attn_t=attn_t,
                               rA=rA, rB=rB, lastq=lastq):
                        attps = state["attps"]
                        # sumexp row to a base-0 SBUF tile (the approx
                        # reciprocal misreads PSUM / non-base-0 inputs)
                        se = small.tile([1, 2 * QB], F32, tag="se", name=f"se{qb}{hp}")
                        au = small.tile([HD, 2 * QB], F32, tag="au", name=f"au{qb}{hp}")
                        r = small.tile([1, 2 * QB], F32, tag="r", name=f"r{qb}{hp}")
                        nc.vector.tensor_copy(se, attps[HD : HD + 1, :])
                        if lastq and hp == 1:
                            # tail: au on the idle Act engine (single-bank
                            # halves), reciprocal on DVE right after se
                            nc.scalar.copy(au[:, 0:QB], attps[0:HD, 0:QB])
                            nc.scalar.copy(au[:, QB : 2 * QB], attps[0:HD, QB : 2 * QB])
                            nc.vector.reciprocal_approx_fast(out=r, in_=se)
                        else:
                            nc.vector.tensor_copy(au, attps[0:HD, :])
                            nc.vector.reciprocal_approx_fast(out=r, in_=se)
                        rbA = small.tile([HD, QB], F32, tag="rbA", name=f"rbA{qb}{hp}")
                        rbB = small.tile([HD, QB], F32, tag="rbB", name=f"rbB{qb}{hp}")
                        nc.gpsimd.partition_broadcast(rbA, r[0:1, 0:QB])
                        nc.gpsimd.partition_broadcast(rbB, r[0:1, QB : 2 * QB])
                        nc.vector.tensor_mul(attn_t[rA, :], au[:, 0:QB], rbA)
                        nc.vector.tensor_mul(attn_t[rB, :], au[:, QB : 2 * QB], rbB)
                        finished.add((qb, hp))

                    pend = []
                    for kt in range(nkt):
                        band = kt >= KTQ * qb
                        qo = (kt - KTQ * qb) * P if band else 0
                        w = QB - qo
                        stt = pst.tile(
                            [P, 2 * QB], F32, tag="st", name=f"st{qb}{hp}{kt}"
                        )
                        ksl = slice(kt * P, (kt + 1) * P)
                        qsl = slice(qb * QB + qo, (qb + 1) * QB)
                        nc.tensor.matmul(
                            stt[:, 0:w],
                            KT[dgt][rA, ksl],
                            QT[dgt][rA, qsl],
                            start=True, stop=True, tile_position=(0, 0),
                        )
                        nc.tensor.matmul(
                            stt[:, QB : QB + w],
                            KT[dgt][rB, ksl],
                            QT[dgt][rB, qsl],
                            start=True, stop=True, tile_position=(64, 0),
                        )
                        ex = work.tile([P, 2 * QB], BF16, tag="ex", name=f"ex{qb}{hp}{kt}")
                        st3 = stt.rearrange("p (h c) -> p h c", c=QB)
                        ex3 = ex.rearrange("p (h c) -> p h c", c=QB)
                        nc.scalar.activation(
                            ex3[:, :, 0:w], st3[:, :, 0:w], EXP, scale=0.125
                        )
                        if band:
                            nc.vector.tensor_mul(
                                ex3[:, :, 0:P], ex3[:, :, 0:P], bm3
                            )
                        pend.append((kt, ex, qo, w))
                        # drain the previous pair's leftovers first so this
                        # pair's exps reach the Act engine without waiting on
                        # the old drain burst
                        if carry:
                            fn, g = carry.pop(0)
                            if fn is None:
                                g()
                            else:
                                fn(g)
                        elif len(pend) > DELAY:
                            attv(pend.pop(0))
                        maybe_filler()
                    for fn, g in carry:
                        if fn is None:
                            g()
                        else:
                            fn(g)
                    carry = []

                    if lastq and hp == 1:
                        for g in pend:
                            attv(g)
                        # prefill first-half out-proj matmuls while the tail
                        # normalize chain runs; borrow freed score-PSUM banks
                        opss = {}
                        for et in range(NET):
                            esl = slice(et * P, (et + 1) * P)
                            if et < 2:
                                opss[et] = pmm.tile(
                                    [P, QB], F32, tag="mm", name=f"pso3{et}"
                                )
                            else:
                                opss[et] = pst.tile(
                                    [P, 2 * QB], F32, tag="st", name=f"pso3{et}"
                                )[:, 0:QB]
                            nc.tensor.matmul(
                                opss[et],
                                wo_t[0][:, esl],
                                attn_tiles[qb][0],
                                start=True,
                                stop=False,
                            )
                        # keep the PE warm through the tail normalize chain
                        # so the final out-proj matmuls run at K=8/8
                        pw = pst.tile([P, 2 * QB], F32, tag="st", name="pwarm")
                        for i in range(36):
                            nc.tensor.matmul(
                                pw[:, 0:QB], dmy[:, 0:P], dmy,
                                start=(i == 0), stop=(i == 35),
                            )
                        nc.vector.tensor_copy(dsink, pw[:, 0:QB])
                        finish()
                    else:
                        carry = [(attv, g) for g in pend] + [(None, finish)]

                for kind, u in fillers:
                    u()
                fillers = []

                if lastq:
                    qsl = slice(qb * QB, (qb + 1) * QB)
                    for et in range(NET):
                        esl = slice(et * P, (et + 1) * P)
                        ops = opss[et]
                        nc.tensor.matmul(
                            ops, wo_t[1][:, esl], attn_tiles[qb][1],
                            start=False, stop=True,
                        )
                        ost = small.tile([P, QB], F32, tag="ost", name=f"ost3{et}")
                        # split evacuation across engines at the tail
                        if et % 2 == 1:
                            nc.scalar.copy(ost, ops)
                        else:
                            nc.vector.tensor_copy(ost, ops)
                        eng = (nc.sync, nc.scalar, nc.gpsimd, nc.sync)[et]
                        eng.dma_start(out=out_d[esl, qsl], in_=ost)

    nc.compile()
    return nc


def _build_nc_generic():
    """Generic-mask variant (reads a host-prepared transposed multiplicative
    mask from DRAM).  Structure identical to the original baseline."""
    nc = bacc.Bacc("TRN2", target_bir_lowering=False, debug=False, num_devices=8)

    xT_d = nc.dram_tensor("xT", [D + 1, S], BF16, kind="ExternalInput").ap()
    wq_d = nc.dram_tensor("wq", [D, DG], BF16, kind="ExternalInput").ap()
    bq_d = nc.dram_tensor("bqv", [DG, 1], F32, kind="ExternalInput").ap()
    wk_d = nc.dram_tensor("wk", [D, DG], BF16, kind="ExternalInput").ap()
    wv_d = nc.dram_tensor("wv", [D + 1, VW], BF16, kind="ExternalInput").ap()
    wo_d = nc.dram_tensor("wo", [DG, D], BF16, kind="ExternalInput").ap()
    mt_d = nc.dram_tensor("mt", [HG, S, S], BF16, kind="ExternalInput").ap()
    out_d = nc.dram_tensor("out", [D, S], F32, kind="ExternalOutput").ap()

    EXP = mybir.ActivationFunctionType.Exp

    with tile.TileContext(nc) as tc:
        with (
            tc.tile_pool(name="consts", bufs=1) as consts,
            tc.tile_pool(name="work", bufs=3) as work,
            tc.tile_pool(name="attn", bufs=3) as attnp,
            tc.tile_pool(name="small", bufs=4) as small,
            tc.tile_pool(name="pmm", bufs=2, space="PSUM") as pmm,
            tc.tile_pool(name="pst", bufs=1, space="PSUM") as pst,
            tc.tile_pool(name="patt", bufs=2, space="PSUM") as patt,
        ):
            xts = [
                consts.tile([P, S], BF16, tag=f"xt{et}", name=f"xts{et}")
                for et in range(NET)
            ]
            wk_t = []
            for et in range(NET):
                tk = consts.tile([P, DG], BF16, tag=f"wk{et}", name=f"wk{et}")
                nc.sync.dma_start(out=tk, in_=wk_d[et * P : (et + 1) * P, :])
                wk_t.append(tk)
            for et in range(NET):
                nc.gpsimd.dma_start(
                    out=xts[et][:, 0:QB], in_=xT_d[et * P : (et + 1) * P, 0:QB]
                )
            wq_t, wv_t = [], []
            for et in range(NET):
                tq = consts.tile([P, DG], BF16, tag=f"wq{et}", name=f"wq{et}")
                nc.scalar.dma_start(out=tq, in_=wq_d[et * P : (et + 1) * P, :])
                wq_t.append(tq)
            bq_sb = []
            for j in range(2):
                t = consts.tile([P, 1], F32, tag=f"bq{j}", name=f"bq{j}")
                nc.scalar.dma_start(out=t, in_=bq_d[j * P : (j + 1) * P, :])
                bq_sb.append(t)
            for et in range(NET):
                tv = consts.tile([P, VW], BF16, tag=f"wv{et}", name=f"wv{et}")
                nc.scalar.dma_start(out=tv, in_=wv_d[et * P : (et + 1) * P, :])
                wv_t.append(tv)
            wvb = consts.tile([1, VW], BF16, tag="wvb", name="wvb")
            nc.scalar.dma_start(out=wvb, in_=wv_d[D : D + 1, :])
            bvb = consts.tile([P, VW], BF16, tag="bvb", name="bvb")
            nc.gpsimd.partition_broadcast(bvb, wvb)
            for sb in range(1, S // QB):
                ssl = slice(sb * QB, (sb + 1) * QB)
                eng = nc.sync if sb == 1 else nc.gpsimd
                for et in range(NET):
                    eng.dma_start(
                        out=xts[et][:, ssl], in_=xT_d[et * P : (et + 1) * P, ssl]
                    )
            wo_t = []
            for j in range(2):
                t = consts.tile([P, D], BF16, tag=f"wo{j}", name=f"wo{j}")
                nc.sync.dma_start(out=t, in_=wo_d[j * P : (j + 1) * P, :])
                wo_t.append(t)

            QT = [consts.tile([P, S], BF16, tag=f"qt{i}", name=f"QT{i}") for i in range(2)]
            KT = [consts.tile([P, S], BF16, tag=f"kt{i}", name=f"KT{i}") for i in range(2)]
            V = [
                consts.tile([P, VW], BF16, tag=f"v{st}", name=f"Vt{st}")
                for st in range(NKT)
            ]
            for sb in range(S // QB):
                ssl = slice(sb * QB, (sb + 1) * QB)
                for dgt in range(2):
                    dsl = slice(dgt * P, (dgt + 1) * P)
                    ps2 = pmm.tile([P, QB], F32, tag="mm", name=f"psk{sb}{dgt}")
                    for et in range(NET):
                        nc.tensor.matmul(
                            ps2,
                            wk_t[et][:, dsl],
                            xts[et][:, ssl],
                            start=(et == 0),
                            stop=(et == NET - 1),
                        )
                    nc.vector.tensor_copy(KT[dgt][:, ssl], ps2)

                    ps = pmm.tile([P, QB], F32, tag="mm", name=f"psq{sb}{dgt}")
                    for et in range(NET):
                        nc.tensor.matmul(
                            ps,
                            wq_t[et][:, dsl],
                            xts[et][:, ssl],
                            start=(et == 0),
                            stop=(et == NET - 1),
                        )
                    nc.vector.tensor_scalar_add(QT[dgt][:, ssl], ps, bq_sb[dgt])

                for st in range(4 * sb, 4 * sb + 4):
                    ksl = slice(st * P, (st + 1) * P)
                    ps = pmm.tile([P, VW], F32, tag="mm", name=f"psv{st}")
                    for et in range(NET):
                        nc.tensor.matmul(
                            ps,
                            xts[et][:, ksl],
                            wv_t[et],
                            start=(et == 0),
                            stop=(et == NET - 1),
                        )
                    nc.vector.tensor_add(V[st], ps, bvb)

            for qb in range(NQB):
                qsl = slice(qb * QB, (qb + 1) * QB)
                attn_t = [attnp.tile([P, QB], BF16, tag=f"attn{i}", name=f"attn{i}_{qb}") for i in range(2)]
                for hp in range(2):
                    hA, hB = 2 * hp, 2 * hp + 1
                    dgt = hp
                    rA, rB = slice(0, HD), slice(HD, 2 * HD)
                    attps = [
                        patt.tile([P, QB], F32, tag="att", name=f"att{qb}_{h}")
                        for h in (hA, hB)
                    ]
                    groups = [
                        [(2 * ip, 0, 0, QB, False), (2 * ip + 1, QB, 0, QB, False)]
                        for ip in range(NKT // 2)
                    ]
                    first_kt = 0
                    last_kt = groups[-1][-1][0]
                    pendings = []
                    for grp in groups:
                        stA = pst.tile([P, 2 * QB], F32, tag="stA", name=f"stA{qb}{hp}{grp[0][0]}")
                        stB = pst.tile([P, 2 * QB], F32, tag="stB", name=f"stB{qb}{hp}{grp[0][0]}")
                        wtot = grp[-1][1] + grp[-1][3]
                        exA = work.tile([P, 2 * QB], BF16, tag="exA", name=f"exA{qb}{hp}{grp[0][0]}")
                        exB = work.tile([P, 2 * QB], BF16, tag="exB", name=f"exB{qb}{hp}{grp[0][0]}")
                        for kt, col, qo, w, _tri in grp:
                            nc.tensor.matmul(
                                stA[:, col : col + w],
                                KT[dgt][rA, kt * P : (kt + 1) * P],
                                QT[dgt][rA, qb * QB + qo : (qb + 1) * QB],
                                start=True, stop=True, tile_position=(0, 0),
                            )
                        nc.scalar.activation(exA[:, :wtot], stA[:, :wtot], EXP, scale=0.125)
                        for kt, col, qo, w, _tri in grp:
                            nc.tensor.matmul(
                                stB[:, col : col + w],
                                KT[dgt][rB, kt * P : (kt + 1) * P],
                                QT[dgt][rB, qb * QB + qo : (qb + 1) * QB],
                                start=True, stop=True, tile_position=(64, 0),
                            )
                        nc.scalar.activation(exB[:, :wtot], stB[:, :wtot], EXP, scale=0.125)
                        for kt, col, qo, w, tri in grp:
                            for h, ex in ((hA, exA), (hB, exB)):
                                mtile = work.tile([P, QB], BF16, tag="mt", name=f"mt{qb}{hp}{kt}{h}")
                                nc.sync.dma_start(
                                    out=mtile,
                                    in_=mt_d[h, kt * P : (kt + 1) * P, qsl],
                                )
                                nc.vector.tensor_mul(
                                    ex[:, col : col + w], ex[:, col : col + w], mtile
                                )
                        pendings.append((grp, exA, exB))
                        if len(pendings) > 2:
                            pgrp, pexA, pexB = pendings.pop(0)
                            for kt, col, qo, w, _tri in pgrp:
                                for h, ex, aps in ((hA, pexA, attps[0]), (hB, pexB, attps[1])):
                                    nc.tensor.matmul(
                                        aps[0 : HD + 1, qo : qo + w],
                                        V[kt][:, h * (HD + 1) : (h + 1) * (HD + 1)],
                                        ex[:, col : col + w],
                                        start=(kt == first_kt), stop=(kt == last_kt),
                                    )
                    for pgrp, pexA, pexB in pendings:
                        for kt, col, qo, w, _tri in pgrp:
                            for h, ex, aps in ((hA, pexA, attps[0]), (hB, pexB, attps[1])):
                                nc.tensor.matmul(
                                    aps[0 : HD + 1, qo : qo + w],
                                    V[kt][:, h * (HD + 1) : (h + 1) * (HD + 1)],
                                    ex[:, col : col + w],
                                    start=(kt == first_kt), stop=(kt == last_kt),
                                )
                    for h, aps, rsl in ((hA, attps[0], rA), (hB, attps[1], rB)):
                        au = small.tile([HD + 32, QB], F32, tag="au", name=f"au{qb}{h}")
                        if qb == NQB - 1 and hp == 1:
                            nc.scalar.copy(au[0 : HD + 1, :], aps[0 : HD + 1, :])
                        else:
                            nc.vector.tensor_copy(au[0 : HD + 1, :], aps[0 : HD + 1, :])
                        t1 = small.tile([32, QB], F32, tag="t1", name=f"t1_{qb}{h}")
                        nc.vector.transpose(t1, au[HD : HD + 32, :])
                        t2 = small.tile([32, QB], F32, tag="t2", name=f"t2_{qb}{h}")
                        nc.vector.reciprocal(
                            out=t2.rearrange("p (j c) -> p j c", c=32)[:, :, 0],
                            in_=t1.rearrange("p (j c) -> p j c", c=32)[:, :, 0],
                        )
                        t3 = small.tile([32, QB], F32, tag="t3", name=f"t3_{qb}{h}")
                        nc.vector.transpose(t3, t2)
                        rb = small.tile([HD, QB], F32, tag="rb", name=f"rb{qb}{h}")
                        nc.gpsimd.partition_broadcast(rb, t3[0:1, :])
                        nc.vector.tensor_mul(attn_t[dgt][rsl, :], au[0:HD, :], rb)

                lastq = qb == NQB - 1
                opss = {}
                for et in range(NET if lastq else 2):
                    esl = slice(et * P, (et + 1) * P)
                    if et < 2:
                        opss[et] = pmm.tile([P, QB], F32, tag="mm", name=f"pso{qb}{et}")
                    else:
                        opss[et] = patt.tile([P, QB], F32, tag="att", name=f"pso{qb}{et}")
                    nc.tensor.matmul(
                        opss[et], wo_t[0][:, esl], attn_t[0], start=True, stop=False
                    )
                for et in range(NET):
                    esl = slice(et * P, (et + 1) * P)
                    if et in opss:
                        ops = opss[et]
                    else:
                        ops = pmm.tile([P, QB], F32, tag="mm", name=f"pso{qb}{et}")
                        nc.tensor.matmul(
                            ops, wo_t[0][:, esl], attn_t[0], start=True, stop=False
                        )
                    nc.tensor.matmul(
                        ops, wo_t[1][:, esl], attn_t[1], start=False, stop=True
                    )
                    ost = work.tile([P, QB], F32, tag="ost", name=f"ost{qb}{et}")
                    if lastq and et % 2 == 1:
                        nc.scalar.copy(ost, ops)
                    else:
                        nc.vector.tensor_copy(ost, ops)
                    nc.sync.dma_start(out=out_d[esl, qsl], in_=ost)

    nc.compile()
    return nc


def _get_nc(causal: bool):
    if causal not in _BUILT:
        _BUILT[causal] = _build_nc_causal() if causal else _build_nc_generic()
    return _BUILT[causal]


def _band_mask():
    """[128, 256] = two copies of the causal triangle (valid iff qi >= ki)."""
    ki = np.arange(P)[:, None]
    qi = np.arange(P)[None, :]
    tri = (qi >= ki).astype(np.float32)
    return np.concatenate([tri, tri], axis=1).astype(NPBF16)


def _prep_core_inputs(x, mask, Wq, bq, Wk, Wv, bv, Wo, causal):
    """Build the 8 per-core input maps (bf16, pre-transposed, biases folded)."""
    ones_row = np.ones((1, S), np.float32)
    bm = _band_mask()
    in_maps = []
    for c in range(8):
        b, hg = c // 2, c % 2
        h0, e0 = hg * HG, hg * DG
        xt = np.concatenate([x[b].T, ones_row], axis=0).astype(NPBF16)
        wq = Wq[e0 : e0 + DG, :].T.astype(NPBF16)
        bqv = np.ascontiguousarray(bq[e0 : e0 + DG][:, None], dtype=np.float32)
        wk = Wk[e0 : e0 + DG, :].T.astype(NPBF16)
        wv = np.zeros((D + 1, VW), np.float32)
        for h in range(HG):
            eh = e0 + h * HD
            wv[:D, h * (HD + 1) : h * (HD + 1) + HD] = Wv[eh : eh + HD, :].T
            wv[D, h * (HD + 1) : h * (HD + 1) + HD] = bv[eh : eh + HD]
            wv[D, h * (HD + 1) + HD] = 1.0
        wo = Wo[:, e0 : e0 + DG].T.astype(NPBF16)
        m = {
            "xT": xt,
            "wq": wq,
            "bqv": bqv,
            "wk": wk,
            "wv": wv.astype(NPBF16),
            "wo": wo,
        }
        if causal:
            m["bm"] = bm
        else:
            mt = np.ascontiguousarray(
                mask[b, h0 : h0 + HG].transpose(0, 2, 1)
            ).astype(NPBF16)
            m["mt"] = mt
        in_maps.append(m)
    return in_maps


def kernel(**inputs):
    from concourse.bass_utils import run_bass_kernel_spmd

    x = np.asarray(inputs["x"], dtype=np.float32)
    mask = np.asarray(inputs["mask"])
    Wq = np.asarray(inputs["Wq"], dtype=np.float32)
    bq = np.asarray(inputs["bq"], dtype=np.float32)
    Wk = np.asarray(inputs["Wk"], dtype=np.float32)
    Wv = np.asarray(inputs["Wv"], dtype=np.float32)
    bv = np.asarray(inputs["bv"], dtype=np.float32)
    Wo = np.asarray(inputs["Wo"], dtype=np.float32)
    bo = np.asarray(inputs["bo"], dtype=np.float32)

    causal = bool(
        (mask == np.tril(np.ones((S, S), dtype=bool))[None, None]).all()
    )

    nc = _get_nc(causal)
    in_maps = _prep_core_inputs(x, mask, Wq, bq, Wk, Wv, bv, Wo, causal)
    res = run_bass_kernel_spmd(nc, in_maps, core_ids=list(range(8)))
    out = np.empty((B, S, D), np.float32)
    for b in range(B):
        partial = res.results[2 * b]["out"] + res.results[2 * b + 1]["out"]
        out[b] = partial.T + bo[None, :]
    return out
